# revision 4
# baseline (speedup 1.0000x reference)
"""MetapathAggrLayer Trainium2 kernel — v4 (measured-cost balanced).

Per node n: e_m = leakyrelu(x[m,n,:].a), w = softmax(e), out = sum_m w_m x[m,n,:].
Data-parallel over N across 8 NeuronCores; nodes-on-partitions layout.

Structure per macro-tile (4096 nodes = 128 partitions x T=32):
one merged fp32 HWDGE load [128, (m t f)]; one merged multiply+prefix-scan
custom DVE op for the scores (segment sums via boundary differences fused
with leakyrelu); softmax pieces spread over Scalar (exp) / GpSimd (sums) /
Vector (reciprocal, w=u*r); weighted sum as broadcast-AP tensor_tensor
multiplies (per-(node,chunk) weight broadcast along F, stride-0 inner dim)
balanced across Vector/GpSimd/Scalar with bf16 products (16-bit adds run
2x on DVE); bf16 accumulate; SWDGE casting store to fp32.

Engine budget per macro (measured per-op costs):
  DVE    scan 8.6 + lrelu/recip/w ~0.7 + t0(3/4) 1.65 + 2 bf16 adds 2.2 ~= 13.2us
  GpSimd h+s 0.5 + t1 3.6 + t3 3.6 + a23 4.0 + swdge-out 1.0       ~= 12.9us
  Scalar exp 0.3 + t2 10.6 + t0(1/4) 2.65                          ~= 13.5us
  DMA    in 4MB + out 1MB                                          ~= 14.0us
"""

import sys

sys.path.insert(0, "/opt/trn_rl_repo")

import numpy as np

import concourse.bacc as bacc
import concourse.mybir as mybir
from concourse import bass_utils, dve_ops
from concourse.dve_spec import Spec, Src0, Src1, C0, scan, maxx, AluOp, lower, _has_src1
from concourse.dve_uop import DveOpSpec
from concourse.tile import TileContext

ALPHA = 0.2
NMETA = 4
F = 64
N_FULL = 1_000_000
N_CORES = 8
T = 32                     # nodes per partition per macro-tile
NODES_PER_MACRO = 128 * T  # 4096
MACROS_PER_CORE = 31
NC_NODES = MACROS_PER_CORE * NODES_PER_MACRO  # 126_976
N_PAD = N_CORES * NC_NODES                    # 1_015_808
NSEG = NMETA * T           # score segments per partition per macro
NM = T * F                 # per-metapath free elems
NALL = NMETA * NM          # merged free elems
T0_DVE = 24                # t0 chunks on Vector; rest on Scalar

_CACHE = {}


def _register_op(name, spec, subdim=False):
    if name in dve_ops._SUB_OPCODE_FOR_NAME:
        return next(o for o in dve_ops.OPS if o.name == name)
    row = dve_ops._CUSTOM_DVE_ROW_BASE + len(dve_ops.OPS)
    assert row < 0x20
    shas = {}
    for ver in ("v3", "v4"):
        s = DveOpSpec(name=name, opcode=row, uops=lower(spec, ver=ver),
                      rd1_en=_has_src1(spec))
        shas[ver] = s.sha(ver)
    op = dve_ops.DveOp(name, spec, subdim, shas)
    dve_ops.OPS.append(op)
    dve_ops.CUSTOM_DVE_SPECS[name] = spec
    dve_ops._SUB_OPCODE_FOR_NAME[name] = row
    return op


def _get_ops():
    scan_mul = _register_op(
        "MPA_SCAN_MUL",
        Spec(
            body=scan(AluOp.ADD, Src0 * Src1),
            reference=lambda in0, in1, s0, s1: np.cumsum(
                (in0.astype(np.float32) * in1.astype(np.float32)), axis=-1
            ),
        ),
    )
    ext_lrelu = _register_op(
        "MPA_EXT_LRELU",
        Spec(
            body=(lambda d: maxx(d, d * C0))(Src0 - Src1),
            reference=lambda in0, in1, s0, s1: np.maximum(in0 - in1, (in0 - in1) * s0),
        ),
    )
    return scan_mul, ext_lrelu


def _build_kernel():
    scan_mul, ext_lrelu = _get_ops()

    nc = bacc.Bacc("TRN2", target_bir_lowering=False, debug=False)
    f32 = mybir.dt.float32
    bf16 = mybir.dt.bfloat16

    x_in = nc.dram_tensor("input", (NMETA, NC_NODES, F), f32, kind="ExternalInput").ap()
    a_rep_in = nc.dram_tensor("a_rep", (128, NM), f32, kind="ExternalInput").ap()
    out = nc.dram_tensor("out", (NC_NODES, F), f32, kind="ExternalOutput").ap()

    mult = mybir.AluOpType.mult
    add = mybir.AluOpType.add

    with TileContext(nc) as tc:
        with tc.tile_pool(name="const", bufs=1) as cpool, \
             tc.tile_pool(name="xp", bufs=2) as xpool, \
             tc.tile_pool(name="pp", bufs=1) as ppool, \
             tc.tile_pool(name="fp", bufs=2) as fpool, \
             tc.tile_pool(name="small", bufs=2) as spool:
            a_rep = cpool.tile([128, NM], f32)
            nc.sync.dma_start(out=a_rep[:, :], in_=a_rep_in)
            a_bc = a_rep[:, :].rearrange("p (o n) -> p o n", o=1).broadcast_to(
                [128, NMETA, NM])

            for i in range(MACROS_PER_CORE):
                lo = i * NODES_PER_MACRO
                hi = lo + NODES_PER_MACRO

                # ---- merged load: [128, (m t f)]
                xm = xpool.tile([128, NALL], f32, tag="x")
                src = x_in[:, lo:hi, :].rearrange("m (p t) f -> p m t f", p=128)
                dst4 = xm[:, :].rearrange("p (m t f) -> p m t f", m=NMETA, f=F)
                nc.sync.dma_start(out=dst4, in_=src)

                # ---- scores: merged multiply+prefix-scan, then boundary
                # differences fused with leakyrelu.
                P = ppool.tile([128, NALL + 1], f32, tag="P")
                nc.gpsimd.memset(P[:, 0:1], 0.0)
                nc.vector._custom_dve(
                    scan_mul, out=P[:, 1:NALL + 1], in0=xm[:, :], in1=a_bc,
                )
                p_hi = P[:, 1:NALL + 1].rearrange("p (s f) -> p s f", f=F)[:, :, F - 1:F]
                p_lo = P[:, 0:NALL].rearrange("p (s f) -> p s f", f=F)[:, :, 0:1]
                e = spool.tile([128, NSEG], f32, tag="e")
                nc.vector._custom_dve(
                    ext_lrelu, out=e[:, :], in0=p_hi, in1=p_lo, s0=ALPHA,
                )

                # ---- softmax over metapaths (m-major segment layout)
                u = spool.tile([128, NSEG], f32, tag="u")
                nc.scalar.activation(u[:, :], e[:, :],
                                     mybir.ActivationFunctionType.Exp)
                h = spool.tile([128, 2 * T], f32, tag="h")
                nc.gpsimd.tensor_tensor(out=h[:, :], in0=u[:, 0:2 * T],
                                        in1=u[:, 2 * T:4 * T], op=add)
                s = spool.tile([128, T], f32, tag="s")
                nc.gpsimd.tensor_tensor(out=s[:, :], in0=h[:, 0:T],
                                        in1=h[:, T:2 * T], op=add)
                r = spool.tile([128, T], f32, tag="r")
                nc.vector.reciprocal(r[:, :], s[:, :])
                w = spool.tile([128, NSEG], f32, tag="w")
                r_bc = r[:, :].rearrange("p (o t) -> p o t", o=1).broadcast_to(
                    [128, NMETA, T])
                nc.vector.tensor_tensor(
                    out=w[:, :].rearrange("p (m t) -> p m t", m=NMETA),
                    in0=u[:, :].rearrange("p (m t) -> p m t", m=NMETA),
                    in1=r_bc, op=mult)

                # ---- weighted sum: per-(node,t) weight broadcast along F
                def wb(m, t0=0, t1=T):
                    return w[:, m * T + t0:m * T + t1].rearrange(
                        "p (t o) -> p t o", o=1).broadcast_to([128, t1 - t0, F])

                def x3(m, t0=0, t1=T):
                    return xm[:, m * NM + t0 * F:m * NM + t1 * F].rearrange(
                        "p (t f) -> p t f", f=F)

                t0 = fpool.tile([128, NM], bf16, tag="t0")
                t1 = fpool.tile([128, NM], bf16, tag="t1")
                t2 = fpool.tile([128, NM], bf16, tag="t2")
                t3 = fpool.tile([128, NM], bf16, tag="t3")
                # m0: Vector (first T0_DVE chunks) + Scalar (rest)
                nc.vector.tensor_tensor(
                    out=t0[:, 0:T0_DVE * F].rearrange("p (t f) -> p t f", f=F),
                    in0=x3(0, 0, T0_DVE), in1=wb(0, 0, T0_DVE), op=mult)
                for t in range(T0_DVE, T):
                    fs = t * F
                    nc.scalar.mul(t0[:, fs:fs + F], xm[:, fs:fs + F],
                                  w[:, t:t + 1])
                # m1, m3: GpSimd
                nc.gpsimd.tensor_tensor(
                    out=t1[:, :].rearrange("p (t f) -> p t f", f=F),
                    in0=x3(1), in1=wb(1), op=mult)
                nc.gpsimd.tensor_tensor(
                    out=t3[:, :].rearrange("p (t f) -> p t f", f=F),
                    in0=x3(3), in1=wb(3), op=mult)
                # m2: Scalar per-t loop
                for t in range(T):
                    fs = t * F
                    nc.scalar.mul(t2[:, fs:fs + F], xm[:, 2 * NM + fs:2 * NM + fs + F],
                                  w[:, 2 * T + t:2 * T + t + 1])

                a01 = fpool.tile([128, NM], bf16, tag="a01")
                a23 = fpool.tile([128, NM], bf16, tag="a23")
                acc = fpool.tile([128, NM], bf16, tag="acc")
                nc.vector.tensor_tensor(out=a01[:, :], in0=t0[:, :], in1=t1[:, :],
                                        op=add)
                nc.gpsimd.tensor_tensor(out=a23[:, :], in0=t2[:, :], in1=t3[:, :],
                                        op=add)
                nc.vector.tensor_tensor(out=acc[:, :], in0=a01[:, :], in1=a23[:, :],
                                        op=add)

                # ---- store with bf16 -> fp32 cast (SWDGE)
                dst = out[lo:hi, :].rearrange("(p t) f -> p (t f)", p=128)
                nc.gpsimd.dma_start(out=dst, in_=acc[:, :])

    nc.compile()
    return nc


def kernel(input, a, _trace=False):
    input = np.ascontiguousarray(np.asarray(input, dtype=np.float32))
    a = np.asarray(a, dtype=np.float32).reshape(F)

    if "nc" not in _CACHE:
        _CACHE["nc"] = _build_kernel()
    nc = _CACHE["nc"]

    pad = N_PAD - input.shape[1]
    xp = np.concatenate(
        [input, np.zeros((NMETA, pad, F), np.float32)], axis=1
    ) if pad else input

    a_rep = np.tile(a[None, :], (128, T)).astype(np.float32)

    in_maps = []
    for c in range(N_CORES):
        sl = xp[:, c * NC_NODES:(c + 1) * NC_NODES, :]
        in_maps.append({"input": np.ascontiguousarray(sl), "a_rep": a_rep})

    res = bass_utils.run_bass_kernel_spmd(
        nc, in_maps, core_ids=list(range(N_CORES)), trace=_trace
    )
    outs = [res.results[c]["out"] for c in range(N_CORES)]
    full = np.concatenate(outs, axis=0)[:N_FULL]
    if _trace:
        return full, res
    return full


# revision 8
# speedup vs baseline: 1.0485x; 1.0485x over previous
"""MetapathAggrLayer Trainium2 kernel — v5 (software-pipelined).

Per node n: e_m = leakyrelu(x[m,n,:].a), w = softmax(e), out = sum_m w_m x[m,n,:].
Data-parallel over N across 8 NeuronCores; nodes-on-partitions layout.

Macro-tile = 4096 nodes (128 partitions x T=32). Stages are skewed across
emission iterations so every engine's in-order queue only sees ready work:

  iter i   Sync : load(i)                       (merged fp32 [128,(m t f)])
           DVE  : scan(i), lrelu(i), a01(i-2), acc(i-2), h/s/recip/w(i), t0(i)
           ACT  : exp(i), t0-tail(i-1), t2(i-1) (per-t scale loop)
           GP   : store(i-3), a23(i-2), t1(i-1), t3(i-1)

Scores: one merged multiply+prefix-scan custom DVE op; segment sums via
boundary differences fused with leakyrelu. Weighted sum: broadcast-AP
tensor_tensor multiplies (weight broadcast along F, stride-0 inner dim),
bf16 products (16-bit contiguous adds run 2x on DVE), SWDGE casting store.

Measured per-op costs give ~13-14us/engine per macro ~= the DMA roofline
(4MB in + 1MB out per macro at ~360GB/s/core).
"""

import sys

sys.path.insert(0, "/opt/trn_rl_repo")

import numpy as np

import concourse.bacc as bacc
import concourse.mybir as mybir
from concourse import bass_utils, dve_ops
from concourse.dve_spec import Spec, Src0, Src1, C0, scan, maxx, AluOp, lower, _has_src1
from concourse.dve_uop import DveOpSpec
from concourse.tile import TileContext

ALPHA = 0.2
NMETA = 4
F = 64
N_FULL = 1_000_000
N_CORES = 8
T = 32                     # nodes per partition per macro-tile
NODES_PER_MACRO = 128 * T  # 4096
MACROS_PER_CORE = 31
NC_NODES = MACROS_PER_CORE * NODES_PER_MACRO  # 126_976
N_PAD = N_CORES * NC_NODES                    # 1_015_808
NSEG = NMETA * T           # score segments per partition per macro
NM = T * F                 # per-metapath free elems
NALL = NMETA * NM          # merged free elems
T0_DVE = 24                # t0 chunks on Vector; rest on Scalar

_CACHE = {}


def _register_op(name, spec, subdim=False):
    if name in dve_ops._SUB_OPCODE_FOR_NAME:
        return next(o for o in dve_ops.OPS if o.name == name)
    row = dve_ops._CUSTOM_DVE_ROW_BASE + len(dve_ops.OPS)
    assert row < 0x20
    shas = {}
    for ver in ("v3", "v4"):
        s = DveOpSpec(name=name, opcode=row, uops=lower(spec, ver=ver),
                      rd1_en=_has_src1(spec))
        shas[ver] = s.sha(ver)
    op = dve_ops.DveOp(name, spec, subdim, shas)
    dve_ops.OPS.append(op)
    dve_ops.CUSTOM_DVE_SPECS[name] = spec
    dve_ops._SUB_OPCODE_FOR_NAME[name] = row
    return op


def _get_ops():
    scan_mul = _register_op(
        "MPA_SCAN_MUL",
        Spec(
            body=scan(AluOp.ADD, Src0 * Src1),
            reference=lambda in0, in1, s0, s1: np.cumsum(
                (in0.astype(np.float32) * in1.astype(np.float32)), axis=-1
            ),
        ),
    )
    ext_lrelu = _register_op(
        "MPA_EXT_LRELU",
        Spec(
            body=(lambda d: maxx(d, d * C0))(Src0 - Src1),
            reference=lambda in0, in1, s0, s1: np.maximum(in0 - in1, (in0 - in1) * s0),
        ),
    )
    return scan_mul, ext_lrelu


def _build_kernel():
    scan_mul, ext_lrelu = _get_ops()

    nc = bacc.Bacc("TRN2", target_bir_lowering=False, debug=False)
    f32 = mybir.dt.float32
    bf16 = mybir.dt.bfloat16

    x_in = nc.dram_tensor("input", (NMETA, NC_NODES, F), f32, kind="ExternalInput").ap()
    a_rep_in = nc.dram_tensor("a_rep", (128, NM), f32, kind="ExternalInput").ap()
    out = nc.dram_tensor("out", (NC_NODES, F), f32, kind="ExternalOutput").ap()

    mult = mybir.AluOpType.mult
    add = mybir.AluOpType.add
    M = MACROS_PER_CORE

    with TileContext(nc) as tc:
        with tc.tile_pool(name="const", bufs=1) as cpool, \
             tc.tile_pool(name="xp", bufs=3) as xpool, \
             tc.tile_pool(name="prod", bufs=2) as prpool, \
             tc.tile_pool(name="comb", bufs=2) as copool, \
             tc.tile_pool(name="small", bufs=2) as spool:
            a_rep = cpool.tile([128, NM], f32)
            nc.sync.dma_start(out=a_rep[:, :], in_=a_rep_in)
            a_bc = a_rep[:, :].rearrange("p (o n) -> p o n", o=1).broadcast_to(
                [128, NMETA, NM])
            P = cpool.tile([128, NALL + 1], f32)
            nc.gpsimd.memset(P[:, 0:1], 0.0)

            tiles = {}

            def wb(d, m, t0=0, t1=T):
                return d["w"][:, m * T + t0:m * T + t1].rearrange(
                    "p (t o) -> p t o", o=1).broadcast_to([128, t1 - t0, F])

            def x3(d, m, t0=0, t1=T):
                return d["xm"][:, m * NM + t0 * F:m * NM + t1 * F].rearrange(
                    "p (t f) -> p t f", f=F)

            for v in range(M + 3):
                # ---- stage A: load + scores + weights + DVE product share
                if v < M:
                    lo = v * NODES_PER_MACRO
                    hi = lo + NODES_PER_MACRO
                    d = tiles[v] = {"lo": lo, "hi": hi}
                    d["xm"] = xpool.tile([128, NALL], f32, tag="x", name="xm")
                    src = x_in[:, lo:hi, :].rearrange("m (p t) f -> p m t f", p=128)
                    dst4 = d["xm"][:, :].rearrange("p (m t f) -> p m t f",
                                                   m=NMETA, f=F)
                    nc.sync.dma_start(out=dst4, in_=src)

                    nc.vector._custom_dve(
                        scan_mul, out=P[:, 1:NALL + 1], in0=d["xm"][:, :], in1=a_bc,
                    )
                    p_hi = P[:, 1:NALL + 1].rearrange(
                        "p (s f) -> p s f", f=F)[:, :, F - 1:F]
                    p_lo = P[:, 0:NALL].rearrange(
                        "p (s f) -> p s f", f=F)[:, :, 0:1]
                    d["e"] = spool.tile([128, NSEG], f32, tag="e", name="e")
                    nc.vector._custom_dve(
                        ext_lrelu, out=d["e"][:, :], in0=p_hi, in1=p_lo, s0=ALPHA,
                    )

                # ---- stage D (iter v-3): casting store
                if 0 <= v - 3:
                    dd = tiles[v - 3]
                    dst = out[dd["lo"]:dd["hi"], :].rearrange(
                        "(p t) f -> p (t f)", p=128)
                    nc.gpsimd.dma_start(out=dst, in_=dd["acc"][:, :])
                    del tiles[v - 3]

                # ---- stage C (iter v-2): combine products
                if 0 <= v - 2 < M:
                    dc = tiles[v - 2]
                    dc["a01"] = copool.tile([128, NM], bf16, tag="a01", name="a01")
                    dc["a23"] = copool.tile([128, NM], bf16, tag="a23", name="a23")
                    dc["acc"] = copool.tile([128, NM], bf16, tag="acc", name="acc")
                    nc.gpsimd.tensor_tensor(out=dc["a23"][:, :], in0=dc["t2"][:, :],
                                            in1=dc["t3"][:, :], op=add)
                    nc.vector.tensor_tensor(out=dc["a01"][:, :], in0=dc["t0"][:, :],
                                            in1=dc["t1"][:, :], op=add)
                    nc.vector.tensor_tensor(out=dc["acc"][:, :], in0=dc["a01"][:, :],
                                            in1=dc["a23"][:, :], op=add)

                # ---- stage A cont.: softmax chain + DVE t0 share + exp
                if v < M:
                    d = tiles[v]
                    d["u"] = spool.tile([128, NSEG], f32, tag="u", name="u")
                    nc.scalar.activation(d["u"][:, :], d["e"][:, :],
                                         mybir.ActivationFunctionType.Exp)
                    d["h"] = spool.tile([128, 2 * T], f32, tag="h", name="h")
                    nc.vector.tensor_tensor(out=d["h"][:, :], in0=d["u"][:, 0:2 * T],
                                            in1=d["u"][:, 2 * T:4 * T], op=add)
                    d["s"] = spool.tile([128, T], f32, tag="s", name="s")
                    nc.vector.tensor_tensor(out=d["s"][:, :], in0=d["h"][:, 0:T],
                                            in1=d["h"][:, T:2 * T], op=add)
                    d["r"] = spool.tile([128, T], f32, tag="r", name="r")
                    nc.vector.reciprocal(d["r"][:, :], d["s"][:, :])
                    d["w"] = spool.tile([128, NSEG], f32, tag="w", name="w")
                    r_bc = d["r"][:, :].rearrange(
                        "p (o t) -> p o t", o=1).broadcast_to([128, NMETA, T])
                    nc.vector.tensor_tensor(
                        out=d["w"][:, :].rearrange("p (m t) -> p m t", m=NMETA),
                        in0=d["u"][:, :].rearrange("p (m t) -> p m t", m=NMETA),
                        in1=r_bc, op=mult)

                    d["t0"] = prpool.tile([128, NM], bf16, tag="t0", name="t0")
                    nc.vector.tensor_tensor(
                        out=d["t0"][:, 0:T0_DVE * F].rearrange(
                            "p (t f) -> p t f", f=F),
                        in0=x3(d, 0, 0, T0_DVE), in1=wb(d, 0, 0, T0_DVE), op=mult)

                # ---- stage B (iter v-1): Scalar + GpSimd product shares
                if 0 <= v - 1 < M:
                    db = tiles[v - 1]
                    db["t1"] = prpool.tile([128, NM], bf16, tag="t1", name="t1")
                    db["t2"] = prpool.tile([128, NM], bf16, tag="t2", name="t2")
                    db["t3"] = prpool.tile([128, NM], bf16, tag="t3", name="t3")
                    for t in range(T0_DVE, T):
                        fs = t * F
                        nc.scalar.mul(db["t0"][:, fs:fs + F], db["xm"][:, fs:fs + F],
                                      db["w"][:, t:t + 1])
                    for t in range(T):
                        fs = t * F
                        nc.scalar.mul(db["t2"][:, fs:fs + F],
                                      db["xm"][:, 2 * NM + fs:2 * NM + fs + F],
                                      db["w"][:, 2 * T + t:2 * T + t + 1])
                    nc.gpsimd.tensor_tensor(
                        out=db["t1"][:, :].rearrange("p (t f) -> p t f", f=F),
                        in0=x3(db, 1), in1=wb(db, 1), op=mult)
                    nc.gpsimd.tensor_tensor(
                        out=db["t3"][:, :].rearrange("p (t f) -> p t f", f=F),
                        in0=x3(db, 3), in1=wb(db, 3), op=mult)

    nc.compile()
    return nc


def kernel(input, a, _trace=False):
    input = np.ascontiguousarray(np.asarray(input, dtype=np.float32))
    a = np.asarray(a, dtype=np.float32).reshape(F)

    if "nc" not in _CACHE:
        _CACHE["nc"] = _build_kernel()
    nc = _CACHE["nc"]

    pad = N_PAD - input.shape[1]
    xp = np.concatenate(
        [input, np.zeros((NMETA, pad, F), np.float32)], axis=1
    ) if pad else input

    a_rep = np.tile(a[None, :], (128, T)).astype(np.float32)

    in_maps = []
    for c in range(N_CORES):
        sl = xp[:, c * NC_NODES:(c + 1) * NC_NODES, :]
        in_maps.append({"input": np.ascontiguousarray(sl), "a_rep": a_rep})

    res = bass_utils.run_bass_kernel_spmd(
        nc, in_maps, core_ids=list(range(N_CORES)), trace=_trace
    )
    outs = [res.results[c]["out"] for c in range(N_CORES)]
    full = np.concatenate(outs, axis=0)[:N_FULL]
    if _trace:
        return full, res
    return full


# revision 13
# speedup vs baseline: 1.0616x; 1.0125x over previous
"""MetapathAggrLayer Trainium2 kernel — v6 (software-pipelined, ACT-first products).

Per node n: e_m = leakyrelu(x[m,n,:].a), w = softmax(e), out = sum_m w_m x[m,n,:].
Data-parallel over N across 8 NeuronCores; nodes-on-partitions layout.

Macro-tile = 4096 nodes (128 partitions x T=32). Stages are skewed across
emission iterations so every engine's in-order queue only sees ready work:

  iter i   Sync : load(i)                       (merged fp32 [128,(m t f)])
           DVE  : scan(i), lrelu(i), a01(i-2), acc(i-2), h/s/recip/w(i), t0(i)
           ACT  : t2(i-1) (per-t scale loop), exp(i), t0-tail(i-1)
           GP   : store(i-3), a23(i-2), t1(i-1), t3(i-1)

ACT runs its product loop at period start (inputs one iter old) so the xm
buffer is released early -- otherwise the next load's WAR wait forces the
period to (t2_end + dma)/bufs ~= 16.7us regardless of engine balance.

Scores: one merged multiply+prefix-scan custom DVE op; segment sums via
boundary differences fused with leakyrelu. Weighted sum: broadcast-AP
tensor_tensor multiplies (weight broadcast along F, stride-0 inner dim),
bf16 products (16-bit contiguous adds run 2x on DVE), SWDGE casting store.

Measured per-op costs give ~13-14us/engine per macro ~= the DMA roofline
(4MB in + 1MB out per macro at ~360GB/s/core).
"""

import sys

sys.path.insert(0, "/opt/trn_rl_repo")

import numpy as np

import concourse.bacc as bacc
import concourse.mybir as mybir
from concourse import bass_utils, dve_ops
from concourse.dve_spec import Spec, Src0, Src1, C0, scan, maxx, AluOp, lower, _has_src1
from concourse.dve_uop import DveOpSpec
from concourse.tile import TileContext

ALPHA = 0.2
NMETA = 4
F = 64
N_FULL = 1_000_000
N_CORES = 8
T = 32                     # nodes per partition per macro-tile
NODES_PER_MACRO = 128 * T  # 4096
MACROS_PER_CORE = 31
NC_NODES = MACROS_PER_CORE * NODES_PER_MACRO  # 126_976
N_PAD = N_CORES * NC_NODES                    # 1_015_808
NSEG = NMETA * T           # score segments per partition per macro
NM = T * F                 # per-metapath free elems
NALL = NMETA * NM          # merged free elems
T0_DVE = 24                # t0 chunks on Vector; rest on Scalar

# Timing-experiment knobs (sim ablations only; defaults = real kernel).
ABL_NOSTORE = False
ABL_NOLOAD = False
ABL_ACT_CHUNKS = None      # override number of t2 chunks on ACT (rest skipped)
ABL_SCAN_DIV = 1           # scan only NALL//k elems
ABL_XBUFS = 3
ABL_PRBUFS = 3
ABL_COBUFS = 2

_CACHE = {}


def _register_op(name, spec, subdim=False):
    if name in dve_ops._SUB_OPCODE_FOR_NAME:
        return next(o for o in dve_ops.OPS if o.name == name)
    row = dve_ops._CUSTOM_DVE_ROW_BASE + len(dve_ops.OPS)
    assert row < 0x20
    shas = {}
    for ver in ("v3", "v4"):
        s = DveOpSpec(name=name, opcode=row, uops=lower(spec, ver=ver),
                      rd1_en=_has_src1(spec))
        shas[ver] = s.sha(ver)
    op = dve_ops.DveOp(name, spec, subdim, shas)
    dve_ops.OPS.append(op)
    dve_ops.CUSTOM_DVE_SPECS[name] = spec
    dve_ops._SUB_OPCODE_FOR_NAME[name] = row
    return op


def _get_ops():
    scan_mul = _register_op(
        "MPA_SCAN_MUL",
        Spec(
            body=scan(AluOp.ADD, Src0 * Src1),
            reference=lambda in0, in1, s0, s1: np.cumsum(
                (in0.astype(np.float32) * in1.astype(np.float32)), axis=-1
            ),
        ),
    )
    ext_lrelu = _register_op(
        "MPA_EXT_LRELU",
        Spec(
            body=(lambda d: maxx(d, d * C0))(Src0 - Src1),
            reference=lambda in0, in1, s0, s1: np.maximum(in0 - in1, (in0 - in1) * s0),
        ),
    )
    return scan_mul, ext_lrelu


def _build_kernel():
    scan_mul, ext_lrelu = _get_ops()

    nc = bacc.Bacc("TRN2", target_bir_lowering=False, debug=False)
    f32 = mybir.dt.float32
    bf16 = mybir.dt.bfloat16

    x_in = nc.dram_tensor("input", (NMETA, NC_NODES, F), f32, kind="ExternalInput").ap()
    a_rep_in = nc.dram_tensor("a_rep", (128, F), f32, kind="ExternalInput").ap()
    out = nc.dram_tensor("out", (NC_NODES, F), f32, kind="ExternalOutput").ap()

    mult = mybir.AluOpType.mult
    add = mybir.AluOpType.add
    M = MACROS_PER_CORE

    with TileContext(nc) as tc:
        with tc.tile_pool(name="const", bufs=1) as cpool, \
             tc.tile_pool(name="xp", bufs=ABL_XBUFS) as xpool, \
             tc.tile_pool(name="prod", bufs=ABL_PRBUFS) as prpool, \
             tc.tile_pool(name="comb", bufs=ABL_COBUFS) as copool, \
             tc.tile_pool(name="small", bufs=3) as spool:
            a_rep = cpool.tile([128, F], f32)
            nc.sync.dma_start(out=a_rep[:, :], in_=a_rep_in)
            a_bc = a_rep[:, :].rearrange("p (o f) -> p o f", o=1).broadcast_to(
                [128, NSEG, F])
            P = cpool.tile([128, NALL + 1], f32)
            nc.gpsimd.memset(P[:, 0:1], 0.0)

            tiles = {}

            def wb(d, m, t0=0, t1=T):
                return d["w"][:, m * T + t0:m * T + t1].rearrange(
                    "p (t o) -> p t o", o=1).broadcast_to([128, t1 - t0, F])

            def x3(d, m, t0=0, t1=T):
                return d["xm"][:, m * NM + t0 * F:m * NM + t1 * F].rearrange(
                    "p (t f) -> p t f", f=F)

            for v in range(M + 3):
                # ---- stage A: load + scores + weights + DVE product share
                if v < M:
                    lo = v * NODES_PER_MACRO
                    hi = lo + NODES_PER_MACRO
                    d = tiles[v] = {"lo": lo, "hi": hi}
                    d["xm"] = xpool.tile([128, NALL], f32, tag="x", name="xm")
                    src = x_in[:, lo:hi, :].rearrange("m (p t) f -> p m t f", p=128)
                    dst4 = d["xm"][:, :].rearrange("p (m t f) -> p m t f",
                                                   m=NMETA, f=F)
                    if ABL_NOLOAD:
                        # small partial load keeps the tile written; timing only
                        nc.sync.dma_start(
                            out=d["xm"][:, 0:128],
                            in_=x_in[0, lo:lo + 256, :].rearrange(
                                "(p t) f -> p (t f)", p=128))
                    else:
                        nc.sync.dma_start(out=dst4, in_=src)

                    NS = NALL // ABL_SCAN_DIV
                    nc.vector._custom_dve(
                        scan_mul, out=P[:, 1:NS + 1], in0=d["xm"][:, 0:NS],
                        in1=a_bc if ABL_SCAN_DIV == 1 else a_rep[:, :].rearrange(
                            "p (o f) -> p o f", o=1).broadcast_to([128, NS // F, F]),
                    )
                    p_hi = P[:, 1:NALL + 1].rearrange(
                        "p (s f) -> p s f", f=F)[:, :, F - 1:F]
                    p_lo = P[:, 0:NALL].rearrange(
                        "p (s f) -> p s f", f=F)[:, :, 0:1]
                    d["e"] = spool.tile([128, NSEG], f32, tag="e", name="e")
                    nc.vector._custom_dve(
                        ext_lrelu, out=d["e"][:, :], in0=p_hi, in1=p_lo, s0=ALPHA,
                    )

                # ---- stage B-act (iter v-1): ACT t2 loop FIRST in the ACT
                # queue so xm(v-1) is released early (see header note)
                if 0 <= v - 1 < M:
                    db = tiles[v - 1]
                    db["t2"] = prpool.tile([128, NM], bf16, tag="t2", name="t2")
                    for t in range(ABL_ACT_CHUNKS if ABL_ACT_CHUNKS is not None else T):
                        fs = t * F
                        nc.scalar.mul(db["t2"][:, fs:fs + F],
                                      db["xm"][:, 2 * NM + fs:2 * NM + fs + F],
                                      db["w"][:, 2 * T + t:2 * T + t + 1])

                # ---- stage D (iter v-3): casting store
                if 0 <= v - 3:
                    dd = tiles[v - 3]
                    if not ABL_NOSTORE:
                        dst = out[dd["lo"]:dd["hi"], :].rearrange(
                            "(p t) f -> p (t f)", p=128)
                        nc.gpsimd.dma_start(out=dst, in_=dd["acc"][:, :])
                    del tiles[v - 3]

                # ---- stage C (iter v-2): combine products
                if 0 <= v - 2 < M:
                    dc = tiles[v - 2]
                    dc["a01"] = copool.tile([128, NM], bf16, tag="a01", name="a01")
                    dc["a23"] = copool.tile([128, NM], bf16, tag="a23", name="a23")
                    nc.gpsimd.tensor_tensor(out=dc["a23"][:, :], in0=dc["t2"][:, :],
                                            in1=dc["t3"][:, :], op=add)
                    nc.vector.tensor_tensor(out=dc["a01"][:, :], in0=dc["t0"][:, :],
                                            in1=dc["t1"][:, :], op=add)
                    nc.vector.tensor_tensor(out=dc["a01"][:, :], in0=dc["a01"][:, :],
                                            in1=dc["a23"][:, :], op=add)
                    dc["acc"] = dc["a01"]

                # ---- stage A cont.: softmax chain + DVE t0 share + exp
                if v < M:
                    d = tiles[v]
                    d["u"] = spool.tile([128, NSEG], f32, tag="u", name="u")
                    nc.scalar.activation(d["u"][:, :], d["e"][:, :],
                                         mybir.ActivationFunctionType.Exp)
                    d["h"] = spool.tile([128, 2 * T], f32, tag="h", name="h")
                    nc.vector.tensor_tensor(out=d["h"][:, :], in0=d["u"][:, 0:2 * T],
                                            in1=d["u"][:, 2 * T:4 * T], op=add)
                    d["s"] = spool.tile([128, T], f32, tag="s", name="s")
                    nc.vector.tensor_tensor(out=d["s"][:, :], in0=d["h"][:, 0:T],
                                            in1=d["h"][:, T:2 * T], op=add)
                    d["r"] = spool.tile([128, T], f32, tag="r", name="r")
                    nc.vector.reciprocal(d["r"][:, :], d["s"][:, :])
                    d["w"] = spool.tile([128, NSEG], f32, tag="w", name="w")
                    r_bc = d["r"][:, :].rearrange(
                        "p (o t) -> p o t", o=1).broadcast_to([128, NMETA, T])
                    nc.vector.tensor_tensor(
                        out=d["w"][:, :].rearrange("p (m t) -> p m t", m=NMETA),
                        in0=d["u"][:, :].rearrange("p (m t) -> p m t", m=NMETA),
                        in1=r_bc, op=mult)

                    d["t0"] = prpool.tile([128, NM], bf16, tag="t0", name="t0")
                    nc.vector.tensor_tensor(
                        out=d["t0"][:, 0:T0_DVE * F].rearrange(
                            "p (t f) -> p t f", f=F),
                        in0=x3(d, 0, 0, T0_DVE), in1=wb(d, 0, 0, T0_DVE), op=mult)

                # ---- stage B rest (iter v-1): ACT t0 tail + GpSimd mults
                if 0 <= v - 1 < M:
                    db = tiles[v - 1]
                    for t in range(T0_DVE, T):
                        fs = t * F
                        nc.scalar.mul(db["t0"][:, fs:fs + F], db["xm"][:, fs:fs + F],
                                      db["w"][:, t:t + 1])
                    db["t1"] = prpool.tile([128, NM], bf16, tag="t1", name="t1")
                    db["t3"] = prpool.tile([128, NM], bf16, tag="t3", name="t3")
                    nc.gpsimd.tensor_tensor(
                        out=db["t1"][:, :].rearrange("p (t f) -> p t f", f=F),
                        in0=x3(db, 1), in1=wb(db, 1), op=mult)
                    nc.gpsimd.tensor_tensor(
                        out=db["t3"][:, :].rearrange("p (t f) -> p t f", f=F),
                        in0=x3(db, 3), in1=wb(db, 3), op=mult)

    nc.compile()
    return nc


def kernel(input, a, _trace=False):
    input = np.ascontiguousarray(np.asarray(input, dtype=np.float32))
    a = np.asarray(a, dtype=np.float32).reshape(F)

    if "nc" not in _CACHE:
        _CACHE["nc"] = _build_kernel()
    nc = _CACHE["nc"]

    pad = N_PAD - input.shape[1]
    xp = np.concatenate(
        [input, np.zeros((NMETA, pad, F), np.float32)], axis=1
    ) if pad else input

    a_rep = np.tile(a[None, :], (128, 1)).astype(np.float32)

    in_maps = []
    for c in range(N_CORES):
        sl = xp[:, c * NC_NODES:(c + 1) * NC_NODES, :]
        in_maps.append({"input": np.ascontiguousarray(sl), "a_rep": a_rep})

    res = bass_utils.run_bass_kernel_spmd(
        nc, in_maps, core_ids=list(range(N_CORES)), trace=_trace
    )
    outs = [res.results[c]["out"] for c in range(N_CORES)]
    full = np.concatenate(outs, axis=0)[:N_FULL]
    if _trace:
        return full, res
    return full
